# revision 46
# baseline (speedup 1.0000x reference)
"""Trainium2 Bass kernel for a single-step attention GRU decoder.

Model (per reference):
    embedded = emb_table[x]                               # [B, E]
    energy   = tanh(enc @ W_w.T + W_b + (h @ U_w.T + U_b)[:, None, :])
    scores   = energy @ V_w[0] + V_b
    alpha    = softmax(scores, axis=S)
    context  = alpha @ enc                                # [B, E]
    GRU single step on [embedded, context] -> h_new       # [B, H]
    prediction = h_new @ fc_w.T + fc_b                    # [B, V]

Sharding (8 NeuronCores):
  - Attention is data-parallel over batch (8 rows/core); the encoder slice
    ships pre-transposed ([E, B_loc*S]) so contractions sit on partitions.
    The U_w@h term (plus biases) is folded into the energy PSUM via a
    selector matmul; softmax+context are pipelined per 512-row chunk.
  - context^T shards are AllGathered (16 KB); the GRU is tensor-parallel
    over hidden dims (full batch, 64 of 512 dims per core), so each core
    loads only 1/8 of W_ih/W_hh.  Gate biases ride the activations'
    per-partition bias port.
  - h_new^T shards are AllGathered; the fc layer is tensor-parallel over
    vocab (all 64 batch rows x 4000 vocab rows per core); the host
    concatenates logit shards.
  - Host-side prep: embedding gather (64 rows), weight transposes,
    sharding.  All FLOPs run on-device; matmuls use fp32r.
"""

import os
import sys

import numpy as np

try:
    import ml_dtypes
except ImportError:  # bf16 numpy dtype
    ml_dtypes = None

if "/opt/trn_rl_repo" not in sys.path:
    sys.path.insert(0, "/opt/trn_rl_repo")

import concourse.bass as bass  # noqa: E402
import concourse.tile as tile  # noqa: E402
from concourse import bacc, mybir  # noqa: E402

F32 = mybir.dt.float32
F32R = mybir.dt.float32r
AF = mybir.ActivationFunctionType
OP = mybir.AluOpType

NCORES = 8
B, S, E, H, A, V = 64, 256, 512, 512, 512, 32000
BL = B // NCORES          # 8 batch rows per core
HL = H // NCORES          # 64 hidden dims per core (GRU shard)
VS = V // NCORES          # 4000 vocab rows per core
R = BL * S                # 2048 attention rows per core
G3 = 3 * H                # 1536
NKT = E // 128            # 4 k-tiles per 512-dim contraction
RC = 512                  # row-chunk (free dim) for the energy matmul
NRC = R // RC             # 4 row chunks
BPC = RC // S             # 2 batch rows per row chunk
FCN = 500                 # fc free-dim chunk
NFC = VS // FCN           # 8 fc chunks


def _declare_io(nc):
    t = {}

    def inp(name, shape, dt=F32R):
        t[name] = nc.dram_tensor(name, list(shape), dt, kind="ExternalInput").ap()

    def outp(name, shape, dt=F32):
        t[name] = nc.dram_tensor(name, list(shape), dt, kind="ExternalOutput").ap()

    inp("encT", (E, R), mybir.dt.bfloat16)      # encoder slice, transposed (bf16)
    inp("wuT", (E, 2 * A), mybir.dt.bfloat16)   # [W_w.T | U_w.T] (bf16)
    inp("vw", (1, A))              # V_w
    inp("wub", (1, A), F32)        # W_b + U_b
    inp("hT", (H, BL), mybir.dt.bfloat16)       # local batch hidden, transposed (bf16)
    inp("hTfull", (H, B))          # full-batch hidden, transposed (GRU gh)
    inp("hTc", (HL, B), F32)       # hidden slice for this core's GRU dims
    inp("embTf", (E, B))           # full embedded rows, transposed
    inp("wihTc", (E + E, 3 * HL))  # W_ih.T cols for this core's dims [1024,192]
    inp("whhTc", (H, 3 * HL))      # W_hh.T cols likewise [512, 192]
    inp("gbias", (HL, 4), F32)     # [r:bih+bhh, z:bih+bhh, n:bih, n:bhh]
    inp("bsel", (BL, R // S * S))  # kron(I8, 1_256): [8, 2048] selector
    inp("id128", (128, 128), F32)  # identity for PE transpose
    inp("fcT", (H, VS), mybir.dt.bfloat16)  # local fc_w slice, transposed (bf16)
    inp("fcb", (1, VS))            # local fc_b slice
    inp("ones", (1, 128))          # ones for K=1 broadcast matmuls

    outp("logits", (B, VS))
    outp("hnewT", (H, B))

    # collective buffers
    t["cc1_in"] = nc.dram_tensor("cc1_in", [E, BL], F32).ap()
    t["cc1_out"] = nc.dram_tensor(
        "cc1_out", [NCORES, E, BL], F32, addr_space="Shared"
    ).ap()
    t["cc2_in"] = nc.dram_tensor("cc2_in", [HL, B], F32).ap()
    t["cc2_out"] = nc.dram_tensor(
        "cc2_out", [H, B], F32, addr_space="Shared"
    ).ap()
    return t


def _gather(nc, kind, t_in, t_out):
    nc.gpsimd.collective_compute(
        "AllGather",
        OP.bypass,
        replica_groups=[list(range(NCORES))],
        ins=[t_in],
        outs=[t_out],
    )


def _emit(nc, tc, t, use_cc=True):
    with (
        tc.tile_pool(name="persist", bufs=1) as pp,
        tc.tile_pool(name="fcstream", bufs=4) as fs,
        tc.tile_pool(name="bias1", bufs=1) as bs,
        tc.tile_pool(name="gru_w", bufs=12) as gws,
    ):
        # ------- small persistent loads first (unblock PE quickly) -------
        wu = [
            pp.tile([128, 2 * A], mybir.dt.bfloat16, tag=f"wu{k}", name=f"wu{k}")
            for k in range(NKT)
        ]
        for k in range(NKT):
            nc.sync.dma_start(out=wu[k], in_=t["wuT"][k * 128 : (k + 1) * 128, :])
        ww = [w[:, :A] for w in wu]
        uw = [w[:, A:] for w in wu]
        vt = pp.tile([128, NKT], F32R, tag="vt")
        nc.sync.dma_start(out=vt, in_=t["vw"].rearrange("o (m p) -> (o p) m", p=128))
        hTt = pp.tile([128, NKT, BL], mybir.dt.bfloat16, tag="hTt")
        nc.sync.dma_start(out=hTt, in_=t["hT"].rearrange("(k p) b -> p k b", p=128))
        onest = pp.tile([1, 128], F32R, tag="ones")
        nc.sync.dma_start(out=onest, in_=t["ones"])
        id128 = pp.tile([128, 128], F32, tag="id128")
        nc.sync.dma_start(out=id128, in_=t["id128"])
        bsel = pp.tile([BL, R], F32R, tag="bsel")
        nc.sync.dma_start(out=bsel, in_=t["bsel"])
        bwu = pp.tile([128, NKT], F32, tag="bwu")
        nc.sync.dma_start(out=bwu, in_=t["wub"].rearrange("o (m p) -> (o p) m", p=128))
        # V_b shifts every score equally -> softmax-invariant; not loaded.

        # ------- bulk loads on the ACT HWDGE queue (few, large) -----------
        enc = [
            pp.tile([128, R], mybir.dt.bfloat16, tag=f"enc{k}", name=f"enc{k}")
            for k in range(NKT)
        ]
        for k in range(NKT):
            nc.scalar.dma_start(
                out=enc[k], in_=t["encT"][k * 128 : (k + 1) * 128, :]
            )
        wih_t, whh_t = [], []
        for k in range(2 * NKT):
            wt_ = gws.tile([128, 3 * HL], F32R, tag="wih", name=f"wih{k}")
            nc.scalar.dma_start(out=wt_, in_=t["wihTc"][k * 128 : (k + 1) * 128, :])
            wih_t.append(wt_)
        for k in range(NKT):
            wt_ = gws.tile([128, 3 * HL], F32R, tag="whh", name=f"whh{k}")
            nc.scalar.dma_start(out=wt_, in_=t["whhTc"][k * 128 : (k + 1) * 128, :])
            whh_t.append(wt_)
        # GRU operands that don't depend on attention
        xk = {}
        for k in range(NKT):
            xt_ = pp.tile([128, B], F32R, tag=f"xk{k}", name=f"xk{k}")
            nc.scalar.dma_start(
                out=xt_, in_=t["embTf"][k * 128 : (k + 1) * 128, :]
            )
            xk[k] = xt_
        hfk = []
        for k in range(NKT):
            ht_ = pp.tile([128, B], F32R, tag=f"hfk{k}", name=f"hfk{k}")
            nc.scalar.dma_start(out=ht_, in_=t["hTfull"][k * 128 : (k + 1) * 128, :])
            hfk.append(ht_)
        gb = pp.tile([HL, 4], F32, tag="gb")
        nc.sync.dma_start(out=gb, in_=t["gbias"])
        hc = pp.tile([HL, B], F32, tag="hc")
        nc.sync.dma_start(out=hc, in_=t["hTc"])
        # fc weights: 4 whole-row loads, sliced per chunk at matmul time
        fct = []
        for k in range(NKT):
            ft = fs.tile([128, VS], mybir.dt.bfloat16, tag="fct", name=f"fct{k}")
            nc.scalar.dma_start(out=ft, in_=t["fcT"][k * 128 : (k + 1) * 128, :])
            fct.append(ft)
        fcbt = bs.tile([1, VS], F32R, tag="fcbt")
        nc.scalar.dma_start(out=fcbt, in_=t["fcb"])

        # ---------------- Uh' = U_w @ h.T + (W_b + U_b), transposed --------
        uh = pp.tile([128, NKT, BL], F32, tag="uh")
        uhT = pp.tile([BL, NKT, 128], F32R, tag="uhT")
        with tc.tile_pool(name="psA", bufs=2, space="PSUM") as psA:
            for m in range(NKT):
                pu = psA.tile([128, BL], F32, tag="pu")
                for k in range(NKT):
                    nc.tensor.matmul(
                        pu[:],
                        uw[k][:, m * 128 : (m + 1) * 128],
                        hTt[:, k, :],
                        start=(k == 0),
                        stop=(k == NKT - 1),
                    )
                nc.vector.tensor_scalar_add(uh[:, m, :], pu[:], bwu[:, m : m + 1])
            for m in range(NKT):
                pt_ = psA.tile([BL, 128], F32, tag="ptu")
                nc.tensor.transpose(pt_[:], uh[:, m, :], id128[:])
                nc.vector.tensor_copy(out=uhT[:, m, :], in_=pt_[:])

        # ---------------- energy / scores / softmax / context, per chunk ---
        ctxT = pp.tile([128, NKT, BL], F32R, tag="ctxT")
        with (
            tc.tile_pool(name="psE", bufs=4, space="PSUM") as psE,
            tc.tile_pool(name="psS", bufs=2, space="PSUM") as psS,
            tc.tile_pool(name="psB", bufs=2, space="PSUM") as psB,
            tc.tile_pool(name="attn_sb", bufs=4) as asb,
            tc.tile_pool(name="soft", bufs=2) as sp,
            tc.tile_pool(name="ctx_sb", bufs=2) as csb,
        ):
            for r in range(NRC):
                rsl = slice(r * RC, (r + 1) * RC)
                ps_s = psS.tile([1, RC], F32, tag="ps_s")
                for m in range(NKT):
                    pe = psE.tile([128, RC], F32, tag="pe")
                    for k in range(NKT):
                        nc.tensor.matmul(
                            pe[:],
                            ww[k][:, m * 128 : (m + 1) * 128],
                            enc[k][:, rsl],
                            start=(k == 0),
                            stop=False,
                        )
                    # += Uh'[a, b] broadcast over s via selector matmul
                    nc.tensor.matmul(
                        pe[:], uhT[:, m, :], bsel[:, rsl], start=False, stop=True
                    )
                    tt = asb.tile([128, RC], F32R, tag="tt")
                    nc.scalar.activation(out=tt, in_=pe[:], func=AF.Tanh)
                    nc.tensor.matmul(
                        ps_s[:],
                        vt[:, m : m + 1],
                        tt[:],
                        start=(m == 0),
                        stop=(m == NKT - 1),
                    )
                # segmented softmax on partition 0 (2 batch rows x 256);
                # per-segment max/sum ride ACT's per-partition bias/scale ports
                s3 = ps_s[:].rearrange("p (b s) -> p b s", b=BPC)
                mx = sp.tile([1, BPC, 1], F32, tag="mx")
                nc.vector.tensor_reduce(
                    out=mx, in_=s3, axis=mybir.AxisListType.X, op=OP.max
                )
                mxn = sp.tile([1, BPC, 1], F32, tag="mxn")
                nc.vector.tensor_scalar_mul(mxn[:], mx[:], -1.0)
                ex = sp.tile([1, BPC, S], F32, tag="ex")
                sm = sp.tile([1, BPC, 1], F32, tag="sm")
                rs = sp.tile([1, BPC, 1], F32, tag="rs")
                alpha = sp.tile([1, BPC, S], F32R, tag="alpha")
                for b in range(BPC):
                    nc.scalar.activation(
                        out=ex[:, b, :], in_=s3[:, b, :], func=AF.Exp,
                        bias=mxn[:, b, :], scale=1.0, accum_out=sm[:, b, :],
                    )
                nc.vector.reciprocal(rs[:], sm[:])
                for b in range(BPC):
                    nc.scalar.mul(alpha[:, b, :], ex[:, b, :], rs[:, b, :])
                # broadcast alpha to all 128 partitions via K=1 ones matmul
                ab = psB.tile([128, RC], F32, tag="ab")
                nc.tensor.matmul(
                    ab[:], onest[:], alpha[:].rearrange("p b s -> p (b s)"),
                    start=True, stop=True,
                )
                # context^T chunk: multiply then segmented reduce
                for k in range(NKT):
                    mt = csb.tile([128, BPC, S], F32, tag="mt")
                    nc.vector.tensor_tensor(
                        out=mt,
                        in0=enc[k][:, rsl].rearrange("p (b s) -> p b s", b=BPC),
                        in1=ab[:].rearrange("p (b s) -> p b s", b=BPC),
                        op=OP.mult,
                    )
                    with nc.allow_low_precision(reason="float32r is 32-bit"):
                        nc.vector.tensor_reduce(
                            out=ctxT[:, k, BPC * r : BPC * (r + 1)],
                            in_=mt[:],
                            axis=mybir.AxisListType.X,
                            op=OP.add,
                        )

        # ---------------- AllGather context^T ----------------
        nc.sync.dma_start(
            out=t["cc1_in"].rearrange("(k p) b -> p k b", p=128),
            in_=ctxT[:].bitcast(F32),
        )
        if use_cc:
            _gather(nc, "cc1", t["cc1_in"], t["cc1_out"])
        else:
            for c in range(NCORES):
                nc.sync.dma_start(out=t["cc1_out"][c], in_=t["cc1_in"])

        # ---------------- GRU: full batch, this core's 64 hidden dims ------
        # gathered context -> xcat^T k-tiles 4..7 in one DMA
        xg = pp.tile([128, NKT, NCORES, BL], F32R, tag="xg")
        for k in range(NKT):
            nc.gpsimd.dma_start(
                out=xg[:, k, :, :],
                in_=t["cc1_out"].rearrange("c (k p) b -> k p c b", p=128)[k],
            )
        for k in range(NKT, 2 * NKT):
            xk[k] = xg[:, k - NKT, :, :]
        hnc = pp.tile([HL, B], F32, tag="hnc")
        with (
            tc.tile_pool(name="psG", bufs=1, space="PSUM") as psG,
            tc.tile_pool(name="gru_sb", bufs=1) as gsb,
        ):
            def gate_psum(tag, g, with_ih, with_hh):
                gsl = slice(g * HL, (g + 1) * HL)
                ps_ = psG.tile([HL, B], F32, tag=tag)
                ops = []
                if with_ih:
                    ops += [(wih_t[k][:, gsl], xk[k][:]) for k in range(NKT)]
                if with_hh:
                    ops += [(whh_t[k][:, gsl], hfk[k][:]) for k in range(NKT)]
                if with_ih:  # gathered context last: overlaps the AllGather
                    ops += [
                        (wih_t[k][:, gsl], xk[k][:]) for k in range(NKT, 2 * NKT)
                    ]
                for i, (lhs, rhs) in enumerate(ops):
                    nc.tensor.matmul(
                        ps_[:], lhs, rhs,
                        start=(i == 0), stop=(i == len(ops) - 1),
                    )
                return ps_

            pr = gate_psum("pr", 0, True, True)
            pz = gate_psum("pz", 1, True, True)
            pi = gate_psum("pi", 2, True, False)
            ph = gate_psum("ph", 2, False, True)

            rg = gsb.tile([HL, B], F32, tag="rg")
            nc.scalar.activation(
                out=rg, in_=pr[:], func=AF.Sigmoid, bias=gb[:, 0:1], scale=1.0
            )
            zg = gsb.tile([HL, B], F32, tag="zg")
            nc.scalar.activation(
                out=zg, in_=pz[:], func=AF.Sigmoid, bias=gb[:, 1:2], scale=1.0
            )
            hn_s = gsb.tile([HL, B], F32, tag="hn_s")
            nc.scalar.activation(
                out=hn_s, in_=ph[:], func=AF.Identity, bias=gb[:, 3:4], scale=1.0
            )
            t1 = gsb.tile([HL, B], F32, tag="t1")
            nc.vector.tensor_tensor(out=t1, in0=rg, in1=hn_s, op=OP.mult)
            t2 = gsb.tile([HL, B], F32, tag="t2")
            nc.vector.tensor_tensor(out=t2, in0=t1, in1=pi[:], op=OP.add)
            ng = gsb.tile([HL, B], F32, tag="ng")
            nc.scalar.activation(
                out=ng, in_=t2, func=AF.Tanh, bias=gb[:, 2:3], scale=1.0
            )
            # h_new = n + z * (h - n)
            dg = gsb.tile([HL, B], F32, tag="dg")
            nc.vector.tensor_tensor(out=dg, in0=hc, in1=ng, op=OP.subtract)
            zd = gsb.tile([HL, B], F32, tag="zd")
            nc.vector.tensor_tensor(out=zd, in0=zg, in1=dg, op=OP.mult)
            nc.vector.tensor_tensor(out=hnc, in0=ng, in1=zd, op=OP.add)

        nc.sync.dma_start(out=t["cc2_in"], in_=hnc[:])
        if use_cc:
            _gather(nc, "cc2", t["cc2_in"], t["cc2_out"])
        else:
            for c in range(NCORES):
                nc.sync.dma_start(
                    out=t["cc2_out"][c * HL : (c + 1) * HL, :], in_=t["cc2_in"]
                )
        nc.sync.dma_start(out=t["hnewT"], in_=t["cc2_out"])

        # ---------------- fc (vocab shard, full batch) ----------------
        with (
            tc.tile_pool(name="psF", bufs=3, space="PSUM") as psF,
            tc.tile_pool(name="fc_sb", bufs=1) as fsb,
        ):
            hg = fsb.tile([128, NKT, B], mybir.dt.bfloat16, tag="hg")
            nc.gpsimd.dma_start(
                out=hg, in_=t["cc2_out"].rearrange("(k p) b -> p k b", p=128)
            )
            hTf = [hg[:, k, :] for k in range(NKT)]
            lgall = fsb.tile([B, VS], F32, tag="lgall")
            for j in range(NFC):
                jsl = slice(j * FCN, (j + 1) * FCN)
                pf = psF.tile([B, FCN], F32, tag="pf")
                nc.tensor.matmul(
                    pf[:], onest[:, :B], fcbt[:, jsl], start=True, stop=False,
                )
                for k in range(NKT):
                    nc.tensor.matmul(
                        pf[:], hTf[k][:], fct[k][:, jsl],
                        start=False, stop=(k == NKT - 1),
                    )
                nc.vector.tensor_copy(out=lgall[:, jsl], in_=pf[:])
            nc.sync.dma_start(out=t["logits"], in_=lgall[:])


def build_program(unroll=1, use_cc=True, num_devices=NCORES):
    nc = bacc.Bacc(
        "TRN2", target_bir_lowering=False, debug=False, num_devices=num_devices
    )
    t = _declare_io(nc)
    if unroll != 1 or not use_cc:
        # Shape-varying dummy input: makes the HLO module signature unique per
        # variant so the NEFF compile cache cannot alias them.
        nc.dram_tensor(
            "utick", [1, unroll + (0 if use_cc else 100)], F32, kind="ExternalInput"
        )
    with tile.TileContext(nc) as tc:
        for _ in range(unroll):
            _emit(nc, tc, t, use_cc=use_cc)
    nc.compile()
    return nc


def prepare_in_maps(inputs):
    """Shard + lay out the full inputs into 8 per-core input dicts."""
    f = np.float32
    x = np.asarray(inputs["x"])
    h = np.asarray(inputs["hidden"], dtype=f)[0]              # [B, H]
    enc = np.asarray(inputs["encoder_outputs"], dtype=f)      # [B, S, E]
    embedded = np.asarray(inputs["emb_table"], dtype=f)[x]    # [B, E]

    def C(a):
        return np.ascontiguousarray(a, dtype=f)

    wihT = np.asarray(inputs["W_ih"], dtype=f).T              # [1024, 1536]
    whhT = np.asarray(inputs["W_hh"], dtype=f).T              # [512, 1536]
    bih = np.asarray(inputs["b_ih"], dtype=f)
    bhh = np.asarray(inputs["b_hh"], dtype=f)
    hT = C(h.T)

    shared = {
        "wuT": np.ascontiguousarray(
            np.concatenate(
                [np.asarray(inputs["W_w"]).T, np.asarray(inputs["U_w"]).T], axis=1
            ).astype(ml_dtypes.bfloat16)
        ),
        "vw": C(np.asarray(inputs["V_w"]).reshape(1, A)),
        "wub": C(
            (np.asarray(inputs["W_b"]) + np.asarray(inputs["U_b"])).reshape(1, A)
        ),
        "hTfull": hT,
        "embTf": C(embedded.T),
        "bsel": C(np.repeat(np.eye(BL, dtype=f), S, axis=1)),
        "id128": np.eye(128, dtype=f),
        "ones": np.ones((1, 128), dtype=f),
    }
    fc_w = np.asarray(inputs["fc_w"], dtype=f)
    fc_b = np.asarray(inputs["fc_b"], dtype=f)

    in_maps = []
    for c in range(NCORES):
        bs_ = slice(c * BL, (c + 1) * BL)
        vs_ = slice(c * VS, (c + 1) * VS)
        hs = [slice(g * H + c * HL, g * H + (c + 1) * HL) for g in range(3)]
        m = dict(shared)
        m["encT"] = np.ascontiguousarray(enc[bs_].reshape(R, E).T.astype(ml_dtypes.bfloat16))
        m["hT"] = np.ascontiguousarray(h[bs_].T.astype(ml_dtypes.bfloat16))
        m["hTc"] = C(hT[c * HL : (c + 1) * HL, :])
        m["wihTc"] = C(np.concatenate([wihT[:, s] for s in hs], axis=1))
        m["whhTc"] = C(np.concatenate([whhT[:, s] for s in hs], axis=1))
        m["gbias"] = C(
            np.stack(
                [
                    bih[hs[0]] + bhh[hs[0]],
                    bih[hs[1]] + bhh[hs[1]],
                    bih[hs[2]],
                    bhh[hs[2]],
                ],
                axis=1,
            )
        )
        m["fcT"] = np.ascontiguousarray(fc_w[vs_].T.astype(ml_dtypes.bfloat16))
        m["fcb"] = C(fc_b[vs_].reshape(1, VS))
        in_maps.append(m)
    return in_maps


def assemble(results):
    logits = np.concatenate([results[c]["logits"] for c in range(NCORES)], axis=1)
    hnew = results[0]["hnewT"].T
    return logits, hnew[None]


_CACHED_NC = None


def kernel(**inputs):
    global _CACHED_NC
    from concourse.bass_utils import run_bass_kernel_spmd

    if _CACHED_NC is None:
        _CACHED_NC = build_program()
    in_maps = prepare_in_maps(inputs)
    res = run_bass_kernel_spmd(_CACHED_NC, in_maps, list(range(NCORES)))
    return assemble(res.results)


# revision 49
# speedup vs baseline: 1.1390x; 1.1390x over previous
"""Trainium2 Bass kernel for a single-step attention GRU decoder.

Model (per reference):
    embedded = emb_table[x]                               # [B, E]
    energy   = tanh(enc @ W_w.T + W_b + (h @ U_w.T + U_b)[:, None, :])
    scores   = energy @ V_w[0] + V_b
    alpha    = softmax(scores, axis=S)
    context  = alpha @ enc                                # [B, E]
    GRU single step on [embedded, context] -> h_new       # [B, H]
    prediction = h_new @ fc_w.T + fc_b                    # [B, V]

Sharding (8 NeuronCores):
  - Attention is data-parallel over batch (8 rows/core); the encoder slice
    ships pre-transposed ([E, B_loc*S]) so contractions sit on partitions.
    The U_w@h term (plus biases) is folded into the energy PSUM via a
    selector matmul; softmax+context are pipelined per 512-row chunk.
  - context^T shards are AllGathered (16 KB); the GRU is tensor-parallel
    over hidden dims (full batch, 64 of 512 dims per core), so each core
    loads only 1/8 of W_ih/W_hh.  Gate biases ride the activations'
    per-partition bias port.
  - h_new^T shards are AllGathered; the fc layer is tensor-parallel over
    vocab (all 64 batch rows x 4000 vocab rows per core); the host
    concatenates logit shards.
  - Host-side prep: embedding gather (64 rows), weight transposes,
    sharding.  All FLOPs run on-device; matmuls use fp32r.
"""

import os
import sys

import numpy as np

try:
    import ml_dtypes
except ImportError:  # bf16 numpy dtype
    ml_dtypes = None

if "/opt/trn_rl_repo" not in sys.path:
    sys.path.insert(0, "/opt/trn_rl_repo")

import concourse.bass as bass  # noqa: E402
import concourse.tile as tile  # noqa: E402
from concourse import bacc, mybir  # noqa: E402

F32 = mybir.dt.float32
F32R = mybir.dt.float32r
AF = mybir.ActivationFunctionType
OP = mybir.AluOpType

NCORES = 8
B, S, E, H, A, V = 64, 256, 512, 512, 512, 32000
BL = B // NCORES          # 8 batch rows per core
HL = H // NCORES          # 64 hidden dims per core (GRU shard)
VS = V // NCORES          # 4000 vocab rows per core
R = BL * S                # 2048 attention rows per core
G3 = 3 * H                # 1536
NKT = E // 128            # 4 k-tiles per 512-dim contraction
RC = 512                  # row-chunk (free dim) for the energy matmul
NRC = R // RC             # 4 row chunks
BPC = RC // S             # 2 batch rows per row chunk
FCN = 500                 # fc free-dim chunk
NFC = VS // FCN           # 8 fc chunks


def _declare_io(nc):
    t = {}

    def inp(name, shape, dt=F32R):
        t[name] = nc.dram_tensor(name, list(shape), dt, kind="ExternalInput").ap()

    def outp(name, shape, dt=F32):
        t[name] = nc.dram_tensor(name, list(shape), dt, kind="ExternalOutput").ap()

    inp("encT", (E, R), mybir.dt.bfloat16)      # encoder slice, transposed (bf16)
    inp("wuT", (E, 2 * A), mybir.dt.bfloat16)   # [W_w.T | U_w.T] (bf16)
    inp("vw", (1, A))              # V_w
    inp("wub", (1, A), F32)        # W_b + U_b
    inp("hT", (H, BL), mybir.dt.bfloat16)       # local batch hidden, transposed (bf16)
    inp("hTfull", (H, B))          # full-batch hidden, transposed (GRU gh)
    inp("hTc", (HL, B), F32)       # hidden slice for this core's GRU dims
    inp("embTf", (E, B))           # full embedded rows, transposed
    inp("wihTc", (E + E, 3 * HL))  # W_ih.T cols for this core's dims [1024,192]
    inp("whhTc", (H, 3 * HL))      # W_hh.T cols likewise [512, 192]
    inp("gbias", (HL, 4), F32)     # [r:bih+bhh, z:bih+bhh, n:bih, n:bhh]
    inp("bsel", (BL, R // S * S))  # kron(I8, 1_256): [8, 2048] selector
    inp("id128", (128, 128), F32)  # identity for PE transpose
    inp("fcT", (H, VS), mybir.dt.bfloat16)  # local fc_w slice, transposed (bf16)
    inp("fcb", (1, VS))            # local fc_b slice
    inp("ones", (1, 128))          # ones for K=1 broadcast matmuls

    outp("logits", (B, VS))
    outp("hnewT", (H, B))

    # collective buffers
    t["cc1_in"] = nc.dram_tensor("cc1_in", [E, BL], F32).ap()
    t["cc1_out"] = nc.dram_tensor(
        "cc1_out", [NCORES, E, BL], F32, addr_space="Shared"
    ).ap()
    t["cc2_in"] = nc.dram_tensor("cc2_in", [HL, B], F32).ap()
    t["cc2_out"] = nc.dram_tensor(
        "cc2_out", [H, B], F32, addr_space="Shared"
    ).ap()
    return t


def _gather(nc, kind, t_in, t_out):
    nc.gpsimd.collective_compute(
        "AllGather",
        OP.bypass,
        replica_groups=[list(range(NCORES))],
        ins=[t_in],
        outs=[t_out],
    )


def _emit(nc, tc, t, use_cc=True):
    with (
        tc.tile_pool(name="persist", bufs=1) as pp,
        tc.tile_pool(name="fcstream", bufs=4) as fs,
        tc.tile_pool(name="bias1", bufs=1) as bs,
        tc.tile_pool(name="gru_w", bufs=12) as gws,
    ):
        # ------- small persistent loads first (unblock PE quickly) -------
        wu = [
            pp.tile([128, 2 * A], mybir.dt.bfloat16, tag=f"wu{k}", name=f"wu{k}")
            for k in range(NKT)
        ]
        for k in range(NKT):
            nc.sync.dma_start(out=wu[k], in_=t["wuT"][k * 128 : (k + 1) * 128, :])
        ww = [w[:, :A] for w in wu]
        uw = [w[:, A:] for w in wu]
        vt = pp.tile([128, NKT], F32R, tag="vt")
        nc.sync.dma_start(out=vt, in_=t["vw"].rearrange("o (m p) -> (o p) m", p=128))
        hTt = pp.tile([128, NKT, BL], mybir.dt.bfloat16, tag="hTt")
        nc.sync.dma_start(out=hTt, in_=t["hT"].rearrange("(k p) b -> p k b", p=128))
        onest = pp.tile([1, 128], F32R, tag="ones")
        nc.sync.dma_start(out=onest, in_=t["ones"])
        id128 = pp.tile([128, 128], F32, tag="id128")
        nc.sync.dma_start(out=id128, in_=t["id128"])
        bsel = pp.tile([BL, R], F32R, tag="bsel")
        nc.sync.dma_start(out=bsel, in_=t["bsel"])
        bwu = pp.tile([128, NKT], F32, tag="bwu")
        nc.sync.dma_start(out=bwu, in_=t["wub"].rearrange("o (m p) -> (o p) m", p=128))
        # V_b shifts every score equally -> softmax-invariant; not loaded.

        # ------- bulk loads on the ACT HWDGE queue (few, large) -----------
        enc = [
            pp.tile([128, R], mybir.dt.bfloat16, tag=f"enc{k}", name=f"enc{k}")
            for k in range(NKT)
        ]
        for k in range(NKT):
            nc.scalar.dma_start(
                out=enc[k], in_=t["encT"][k * 128 : (k + 1) * 128, :]
            )
        wih_t, whh_t = [], []
        for k in range(2 * NKT):
            wt_ = gws.tile([128, 3 * HL], F32R, tag="wih", name=f"wih{k}")
            nc.scalar.dma_start(out=wt_, in_=t["wihTc"][k * 128 : (k + 1) * 128, :])
            wih_t.append(wt_)
        for k in range(NKT):
            wt_ = gws.tile([128, 3 * HL], F32R, tag="whh", name=f"whh{k}")
            nc.scalar.dma_start(out=wt_, in_=t["whhTc"][k * 128 : (k + 1) * 128, :])
            whh_t.append(wt_)
        # GRU operands that don't depend on attention
        xk = {}
        for k in range(NKT):
            xt_ = pp.tile([128, B], F32R, tag=f"xk{k}", name=f"xk{k}")
            nc.scalar.dma_start(
                out=xt_, in_=t["embTf"][k * 128 : (k + 1) * 128, :]
            )
            xk[k] = xt_
        hfk = []
        for k in range(NKT):
            ht_ = pp.tile([128, B], F32R, tag=f"hfk{k}", name=f"hfk{k}")
            nc.scalar.dma_start(out=ht_, in_=t["hTfull"][k * 128 : (k + 1) * 128, :])
            hfk.append(ht_)
        gb = pp.tile([HL, 4], F32, tag="gb")
        nc.sync.dma_start(out=gb, in_=t["gbias"])
        hc = pp.tile([HL, B], F32, tag="hc")
        nc.sync.dma_start(out=hc, in_=t["hTc"])
        # fc weights: 4 whole-row loads, sliced per chunk at matmul time
        fct = []
        for k in range(NKT):
            ft = fs.tile([128, VS], mybir.dt.bfloat16, tag="fct", name=f"fct{k}")
            nc.scalar.dma_start(out=ft, in_=t["fcT"][k * 128 : (k + 1) * 128, :])
            fct.append(ft)
        fcbt = bs.tile([1, VS], F32R, tag="fcbt")
        nc.scalar.dma_start(out=fcbt, in_=t["fcb"])

        # ---------------- Uh' = U_w @ h.T + (W_b + U_b), transposed --------
        uh = pp.tile([128, NKT, BL], F32, tag="uh")
        uhT = pp.tile([BL, NKT, 128], F32R, tag="uhT")
        with tc.tile_pool(name="psA", bufs=2, space="PSUM") as psA:
            for m in range(NKT):
                pu = psA.tile([128, BL], F32, tag="pu")
                for k in range(NKT):
                    nc.tensor.matmul(
                        pu[:],
                        uw[k][:, m * 128 : (m + 1) * 128],
                        hTt[:, k, :],
                        start=(k == 0),
                        stop=(k == NKT - 1),
                    )
                nc.vector.tensor_scalar_add(uh[:, m, :], pu[:], bwu[:, m : m + 1])
            for m in range(NKT):
                pt_ = psA.tile([BL, 128], F32, tag="ptu")
                nc.tensor.transpose(pt_[:], uh[:, m, :], id128[:])
                nc.vector.tensor_copy(out=uhT[:, m, :], in_=pt_[:])

        # ---------------- energy / scores / softmax / context, per chunk ---
        ctxT = pp.tile([128, NKT, BL], F32R, tag="ctxT")
        with (
            tc.tile_pool(name="psE", bufs=2, space="PSUM") as psE,
            tc.tile_pool(name="psS", bufs=1, space="PSUM") as psS,
            tc.tile_pool(name="psB", bufs=1, space="PSUM") as psB,
            tc.tile_pool(name="attn_sb", bufs=4) as asb,
            tc.tile_pool(name="soft", bufs=2) as sp,
            tc.tile_pool(name="ctx_sb", bufs=2) as csb,
        ):
            for r in range(NRC):
                rsl = slice(r * RC, (r + 1) * RC)
                ps_s = psS.tile([1, RC], F32, tag="ps_s")
                for m in range(NKT):
                    pe = psE.tile([128, RC], F32, tag="pe")
                    for k in range(NKT):
                        nc.tensor.matmul(
                            pe[:],
                            ww[k][:, m * 128 : (m + 1) * 128],
                            enc[k][:, rsl],
                            start=(k == 0),
                            stop=False,
                        )
                    # += Uh'[a, b] broadcast over s via selector matmul
                    nc.tensor.matmul(
                        pe[:], uhT[:, m, :], bsel[:, rsl], start=False, stop=True
                    )
                    tt = asb.tile([128, RC], F32R, tag="tt")
                    nc.scalar.activation(out=tt, in_=pe[:], func=AF.Tanh)
                    nc.tensor.matmul(
                        ps_s[:],
                        vt[:, m : m + 1],
                        tt[:],
                        start=(m == 0),
                        stop=(m == NKT - 1),
                    )
                # segmented softmax on partition 0 (2 batch rows x 256);
                # per-segment max/sum ride ACT's per-partition bias/scale ports
                s3 = ps_s[:].rearrange("p (b s) -> p b s", b=BPC)
                mx = sp.tile([1, BPC, 1], F32, tag="mx")
                nc.vector.tensor_reduce(
                    out=mx, in_=s3, axis=mybir.AxisListType.X, op=OP.max
                )
                mxn = sp.tile([1, BPC, 1], F32, tag="mxn")
                nc.vector.tensor_scalar_mul(mxn[:], mx[:], -1.0)
                ex = sp.tile([1, BPC, S], F32, tag="ex")
                sm = sp.tile([1, BPC, 1], F32, tag="sm")
                rs = sp.tile([1, BPC, 1], F32, tag="rs")
                alpha = sp.tile([1, BPC, S], F32R, tag="alpha")
                for b in range(BPC):
                    nc.scalar.activation(
                        out=ex[:, b, :], in_=s3[:, b, :], func=AF.Exp,
                        bias=mxn[:, b, :], scale=1.0, accum_out=sm[:, b, :],
                    )
                nc.vector.reciprocal(rs[:], sm[:])
                for b in range(BPC):
                    nc.scalar.mul(alpha[:, b, :], ex[:, b, :], rs[:, b, :])
                # broadcast alpha to all 128 partitions via K=1 ones matmul
                ab = psB.tile([128, RC], F32, tag="ab")
                nc.tensor.matmul(
                    ab[:], onest[:], alpha[:].rearrange("p b s -> p (b s)"),
                    start=True, stop=True,
                )
                # context^T chunk: multiply then segmented reduce
                for k in range(NKT):
                    mt = csb.tile([128, BPC, S], F32, tag="mt")
                    nc.vector.tensor_tensor(
                        out=mt,
                        in0=enc[k][:, rsl].rearrange("p (b s) -> p b s", b=BPC),
                        in1=ab[:].rearrange("p (b s) -> p b s", b=BPC),
                        op=OP.mult,
                    )
                    with nc.allow_low_precision(reason="float32r is 32-bit"):
                        nc.vector.tensor_reduce(
                            out=ctxT[:, k, BPC * r : BPC * (r + 1)],
                            in_=mt[:],
                            axis=mybir.AxisListType.X,
                            op=OP.add,
                        )

        # ---------------- AllGather context^T ----------------
        nc.sync.dma_start(
            out=t["cc1_in"].rearrange("(k p) b -> p k b", p=128),
            in_=ctxT[:].bitcast(F32),
        )
        if use_cc:
            _gather(nc, "cc1", t["cc1_in"], t["cc1_out"])
        else:
            for c in range(NCORES):
                nc.sync.dma_start(out=t["cc1_out"][c], in_=t["cc1_in"])
        # ---------------- GRU: full batch, this core's 64 hidden dims ------
        hnc = pp.tile([HL, B], F32, tag="hnc")
        with (
            tc.tile_pool(name="psG", bufs=1, space="PSUM") as psG,
            tc.tile_pool(name="gru_sb", bufs=1) as gsb,
        ):
            def gate_pre(tag, g, with_ih, with_hh, stop_here):
                """Embedded + hidden matmuls: no dependency on the gather."""
                gsl = slice(g * HL, (g + 1) * HL)
                ps_ = psG.tile([HL, B], F32, tag=tag, name=tag)[:]
                ops = []
                if with_ih:
                    ops += [(wih_t[k][:, gsl], xk[k][:]) for k in range(NKT)]
                if with_hh:
                    ops += [(whh_t[k][:, gsl], hfk[k][:]) for k in range(NKT)]
                for i, (lhs, rhs) in enumerate(ops):
                    nc.tensor.matmul(
                        ps_, lhs, rhs,
                        start=(i == 0),
                        stop=(stop_here and i == len(ops) - 1),
                    )
                return ps_

            pr = gate_pre("pr", 0, True, True, False)
            pz = gate_pre("pz", 1, True, True, False)
            pi = gate_pre("pi", 2, True, False, False)
            ph = gate_pre("ph", 2, False, True, True)

            # gathered context -> xcat^T k-tiles 4..7
            xg = pp.tile([128, NKT, NCORES, BL], F32R, tag="xg")
            for k in range(NKT):
                nc.gpsimd.dma_start(
                    out=xg[:, k, :, :],
                    in_=t["cc1_out"].rearrange("c (k p) b -> k p c b", p=128)[k],
                )
            for k in range(NKT, 2 * NKT):
                xk[k] = xg[:, k - NKT, :, :]
            for g, ps_ in ((0, pr), (1, pz), (2, pi)):
                gsl = slice(g * HL, (g + 1) * HL)
                for k in range(NKT, 2 * NKT):
                    nc.tensor.matmul(
                        ps_, wih_t[k][:, gsl], xk[k][:],
                        start=False, stop=(k == 2 * NKT - 1),
                    )

            rg = gsb.tile([HL, B], F32, tag="rg")
            nc.scalar.activation(
                out=rg, in_=pr, func=AF.Sigmoid, bias=gb[:, 0:1], scale=1.0
            )
            zg = gsb.tile([HL, B], F32, tag="zg")
            nc.scalar.activation(
                out=zg, in_=pz, func=AF.Sigmoid, bias=gb[:, 1:2], scale=1.0
            )
            hn_s = gsb.tile([HL, B], F32, tag="hn_s")
            nc.scalar.activation(
                out=hn_s, in_=ph, func=AF.Identity, bias=gb[:, 3:4], scale=1.0
            )
            t1 = gsb.tile([HL, B], F32, tag="t1")
            nc.vector.tensor_tensor(out=t1, in0=rg, in1=hn_s, op=OP.mult)
            t2 = gsb.tile([HL, B], F32, tag="t2")
            nc.vector.tensor_tensor(out=t2, in0=t1, in1=pi, op=OP.add)
            ng = gsb.tile([HL, B], F32, tag="ng")
            nc.scalar.activation(
                out=ng, in_=t2, func=AF.Tanh, bias=gb[:, 2:3], scale=1.0
            )
            # h_new = n + z * (h - n)
            dg = gsb.tile([HL, B], F32, tag="dg")
            nc.vector.tensor_tensor(out=dg, in0=hc, in1=ng, op=OP.subtract)
            zd = gsb.tile([HL, B], F32, tag="zd")
            nc.vector.tensor_tensor(out=zd, in0=zg, in1=dg, op=OP.mult)
            nc.vector.tensor_tensor(out=hnc, in0=ng, in1=zd, op=OP.add)

        nc.sync.dma_start(out=t["cc2_in"], in_=hnc[:])
        if use_cc:
            _gather(nc, "cc2", t["cc2_in"], t["cc2_out"])
        else:
            for c in range(NCORES):
                nc.sync.dma_start(
                    out=t["cc2_out"][c * HL : (c + 1) * HL, :], in_=t["cc2_in"]
                )
        nc.sync.dma_start(out=t["hnewT"], in_=t["cc2_out"])

        # ---------------- fc (vocab shard, full batch) ----------------
        with (
            tc.tile_pool(name="psF", bufs=3, space="PSUM") as psF,
            tc.tile_pool(name="fc_sb", bufs=1) as fsb,
        ):
            hg = fsb.tile([128, NKT, B], mybir.dt.bfloat16, tag="hg")
            nc.gpsimd.dma_start(
                out=hg, in_=t["cc2_out"].rearrange("(k p) b -> p k b", p=128)
            )
            hTf = [hg[:, k, :] for k in range(NKT)]
            lgall = fsb.tile([B, VS], F32, tag="lgall")
            for j in range(NFC):
                jsl = slice(j * FCN, (j + 1) * FCN)
                pf = psF.tile([B, FCN], F32, tag="pf")
                nc.tensor.matmul(
                    pf[:], onest[:, :B], fcbt[:, jsl], start=True, stop=False,
                )
                for k in range(NKT):
                    nc.tensor.matmul(
                        pf[:], hTf[k][:], fct[k][:, jsl],
                        start=False, stop=(k == NKT - 1),
                    )
                nc.vector.tensor_copy(out=lgall[:, jsl], in_=pf[:])
            nc.sync.dma_start(out=t["logits"], in_=lgall[:])


def build_program(unroll=1, use_cc=True, num_devices=NCORES):
    nc = bacc.Bacc(
        "TRN2", target_bir_lowering=False, debug=False, num_devices=num_devices
    )
    t = _declare_io(nc)
    if unroll != 1 or not use_cc:
        # Shape-varying dummy input: makes the HLO module signature unique per
        # variant so the NEFF compile cache cannot alias them.
        nc.dram_tensor(
            "utick", [1, unroll + (0 if use_cc else 100)], F32, kind="ExternalInput"
        )
    with tile.TileContext(nc) as tc:
        for _ in range(unroll):
            _emit(nc, tc, t, use_cc=use_cc)
    nc.compile()
    return nc


def prepare_in_maps(inputs):
    """Shard + lay out the full inputs into 8 per-core input dicts."""
    f = np.float32
    x = np.asarray(inputs["x"])
    h = np.asarray(inputs["hidden"], dtype=f)[0]              # [B, H]
    enc = np.asarray(inputs["encoder_outputs"], dtype=f)      # [B, S, E]
    embedded = np.asarray(inputs["emb_table"], dtype=f)[x]    # [B, E]

    def C(a):
        return np.ascontiguousarray(a, dtype=f)

    wihT = np.asarray(inputs["W_ih"], dtype=f).T              # [1024, 1536]
    whhT = np.asarray(inputs["W_hh"], dtype=f).T              # [512, 1536]
    bih = np.asarray(inputs["b_ih"], dtype=f)
    bhh = np.asarray(inputs["b_hh"], dtype=f)
    hT = C(h.T)

    shared = {
        "wuT": np.ascontiguousarray(
            np.concatenate(
                [np.asarray(inputs["W_w"]).T, np.asarray(inputs["U_w"]).T], axis=1
            ).astype(ml_dtypes.bfloat16)
        ),
        "vw": C(np.asarray(inputs["V_w"]).reshape(1, A)),
        "wub": C(
            (np.asarray(inputs["W_b"]) + np.asarray(inputs["U_b"])).reshape(1, A)
        ),
        "hTfull": hT,
        "embTf": C(embedded.T),
        "bsel": C(np.repeat(np.eye(BL, dtype=f), S, axis=1)),
        "id128": np.eye(128, dtype=f),
        "ones": np.ones((1, 128), dtype=f),
    }
    fc_w = np.asarray(inputs["fc_w"], dtype=f)
    fc_b = np.asarray(inputs["fc_b"], dtype=f)

    in_maps = []
    for c in range(NCORES):
        bs_ = slice(c * BL, (c + 1) * BL)
        vs_ = slice(c * VS, (c + 1) * VS)
        hs = [slice(g * H + c * HL, g * H + (c + 1) * HL) for g in range(3)]
        m = dict(shared)
        m["encT"] = np.ascontiguousarray(enc[bs_].reshape(R, E).T.astype(ml_dtypes.bfloat16))
        m["hT"] = np.ascontiguousarray(h[bs_].T.astype(ml_dtypes.bfloat16))
        m["hTc"] = C(hT[c * HL : (c + 1) * HL, :])
        m["wihTc"] = C(np.concatenate([wihT[:, s] for s in hs], axis=1))
        m["whhTc"] = C(np.concatenate([whhT[:, s] for s in hs], axis=1))
        m["gbias"] = C(
            np.stack(
                [
                    bih[hs[0]] + bhh[hs[0]],
                    bih[hs[1]] + bhh[hs[1]],
                    bih[hs[2]],
                    bhh[hs[2]],
                ],
                axis=1,
            )
        )
        m["fcT"] = np.ascontiguousarray(fc_w[vs_].T.astype(ml_dtypes.bfloat16))
        m["fcb"] = C(fc_b[vs_].reshape(1, VS))
        in_maps.append(m)
    return in_maps


def assemble(results):
    logits = np.concatenate([results[c]["logits"] for c in range(NCORES)], axis=1)
    hnew = results[0]["hnewT"].T
    return logits, hnew[None]


_CACHED_NC = None


def kernel(**inputs):
    global _CACHED_NC
    from concourse.bass_utils import run_bass_kernel_spmd

    if _CACHED_NC is None:
        _CACHED_NC = build_program()
    in_maps = prepare_in_maps(inputs)
    res = run_bass_kernel_spmd(_CACHED_NC, in_maps, list(range(NCORES)))
    return assemble(res.results)


# revision 52
# speedup vs baseline: 1.1575x; 1.0162x over previous
"""Trainium2 Bass kernel for a single-step attention GRU decoder.

Model (per reference):
    embedded = emb_table[x]                               # [B, E]
    energy   = tanh(enc @ W_w.T + W_b + (h @ U_w.T + U_b)[:, None, :])
    scores   = energy @ V_w[0] + V_b
    alpha    = softmax(scores, axis=S)
    context  = alpha @ enc                                # [B, E]
    GRU single step on [embedded, context] -> h_new       # [B, H]
    prediction = h_new @ fc_w.T + fc_b                    # [B, V]

Sharding (8 NeuronCores):
  - Attention is data-parallel over batch (8 rows/core); the encoder slice
    ships pre-transposed ([E, B_loc*S]) so contractions sit on partitions.
    The U_w@h term (plus biases) is folded into the energy PSUM via a
    selector matmul; softmax+context are pipelined per 512-row chunk.
  - context^T shards are AllGathered (16 KB); the GRU is tensor-parallel
    over hidden dims (full batch, 64 of 512 dims per core), so each core
    loads only 1/8 of W_ih/W_hh.  Gate biases ride the activations'
    per-partition bias port.
  - h_new^T shards are AllGathered; the fc layer is tensor-parallel over
    vocab (all 64 batch rows x 4000 vocab rows per core); the host
    concatenates logit shards.
  - Host-side prep: embedding gather (64 rows), weight transposes,
    sharding.  All FLOPs run on-device; matmuls use fp32r.
"""

import os
import sys

import numpy as np

try:
    import ml_dtypes
except ImportError:  # bf16 numpy dtype
    ml_dtypes = None

if "/opt/trn_rl_repo" not in sys.path:
    sys.path.insert(0, "/opt/trn_rl_repo")

import concourse.bass as bass  # noqa: E402
import concourse.tile as tile  # noqa: E402
from concourse import bacc, mybir  # noqa: E402

F32 = mybir.dt.float32
F32R = mybir.dt.float32r
AF = mybir.ActivationFunctionType
OP = mybir.AluOpType

NCORES = 8
B, S, E, H, A, V = 64, 256, 512, 512, 512, 32000
BL = B // NCORES          # 8 batch rows per core
HL = H // NCORES          # 64 hidden dims per core (GRU shard)
VS = V // NCORES          # 4000 vocab rows per core
R = BL * S                # 2048 attention rows per core
G3 = 3 * H                # 1536
NKT = E // 128            # 4 k-tiles per 512-dim contraction
RC = 512                  # row-chunk (free dim) for the energy matmul
NRC = R // RC             # 4 row chunks
BPC = RC // S             # 2 batch rows per row chunk
FCN = 500                 # fc free-dim chunk
NFC = VS // FCN           # 8 fc chunks


def _declare_io(nc):
    t = {}

    def inp(name, shape, dt=F32R):
        t[name] = nc.dram_tensor(name, list(shape), dt, kind="ExternalInput").ap()

    def outp(name, shape, dt=F32):
        t[name] = nc.dram_tensor(name, list(shape), dt, kind="ExternalOutput").ap()

    inp("encT", (E, R), mybir.dt.bfloat16)      # encoder slice, transposed (bf16)
    inp("wuT", (E, 2 * A), mybir.dt.bfloat16)   # [W_w.T | U_w.T] (bf16)
    inp("vw", (1, A))              # V_w
    inp("wub", (1, A), F32)        # W_b + U_b
    inp("hT", (H, BL), mybir.dt.bfloat16)       # local batch hidden, transposed (bf16)
    inp("hTfull", (H, B))          # full-batch hidden, transposed (GRU gh)
    inp("hTc", (HL, B), F32)       # hidden slice for this core's GRU dims
    inp("embTf", (E, B))           # full embedded rows, transposed
    inp("wihTc", (E + E, 3 * HL))  # W_ih.T cols for this core's dims [1024,192]
    inp("whhTc", (H, 3 * HL))      # W_hh.T cols likewise [512, 192]
    inp("gbias", (HL, 4), F32)     # [r:bih+bhh, z:bih+bhh, n:bih, n:bhh]
    inp("bsel", (BL, R // S * S))  # kron(I8, 1_256): [8, 2048] selector
    inp("id128", (128, 128), F32)  # identity for PE transpose
    inp("fcT", (H, VS), mybir.dt.bfloat16)  # local fc_w slice, transposed (bf16)
    inp("fcb", (1, VS))            # local fc_b slice
    inp("ones", (1, 128))          # ones for K=1 broadcast matmuls

    outp("logits", (B, VS))
    outp("hnewT", (H, B))

    # collective buffers
    t["cc1_in"] = nc.dram_tensor("cc1_in", [E, BL], F32).ap()
    t["cc1_out"] = nc.dram_tensor(
        "cc1_out", [NCORES, E, BL], F32, addr_space="Shared"
    ).ap()
    t["cc2_in"] = nc.dram_tensor("cc2_in", [HL, B], F32).ap()
    t["cc2_out"] = nc.dram_tensor(
        "cc2_out", [H, B], F32, addr_space="Shared"
    ).ap()
    return t


def _gather(nc, kind, t_in, t_out):
    nc.gpsimd.collective_compute(
        "AllGather",
        OP.bypass,
        replica_groups=[list(range(NCORES))],
        ins=[t_in],
        outs=[t_out],
    )


def _emit(nc, tc, t, use_cc=True):
    with (
        tc.tile_pool(name="persist", bufs=1) as pp,
        tc.tile_pool(name="fcstream", bufs=4) as fs,
        tc.tile_pool(name="bias1", bufs=1) as bs,
        tc.tile_pool(name="gru_w", bufs=12) as gws,
    ):
        # ------- small persistent loads first (unblock PE quickly) -------
        wu = [
            pp.tile([128, 2 * A], mybir.dt.bfloat16, tag=f"wu{k}", name=f"wu{k}")
            for k in range(NKT)
        ]
        for k in range(NKT):
            nc.sync.dma_start(out=wu[k], in_=t["wuT"][k * 128 : (k + 1) * 128, :])
        ww = [w[:, :A] for w in wu]
        uw = [w[:, A:] for w in wu]
        vt = pp.tile([128, NKT], F32R, tag="vt")
        nc.sync.dma_start(out=vt, in_=t["vw"].rearrange("o (m p) -> (o p) m", p=128))
        hTt = pp.tile([128, NKT, BL], mybir.dt.bfloat16, tag="hTt")
        nc.sync.dma_start(out=hTt, in_=t["hT"].rearrange("(k p) b -> p k b", p=128))
        onest = pp.tile([1, 128], F32R, tag="ones")
        nc.sync.dma_start(out=onest, in_=t["ones"])
        id128 = pp.tile([128, 128], F32, tag="id128")
        nc.sync.dma_start(out=id128, in_=t["id128"])
        bsel = pp.tile([BL, R], F32R, tag="bsel")
        nc.sync.dma_start(out=bsel, in_=t["bsel"])
        bwu = pp.tile([128, NKT], F32, tag="bwu")
        nc.sync.dma_start(out=bwu, in_=t["wub"].rearrange("o (m p) -> (o p) m", p=128))
        # V_b shifts every score equally -> softmax-invariant; not loaded.

        # ------- bulk loads on the ACT HWDGE queue (few, large) -----------
        enc = [
            pp.tile([128, R], mybir.dt.bfloat16, tag=f"enc{k}", name=f"enc{k}")
            for k in range(NKT)
        ]
        for k in range(NKT):
            nc.scalar.dma_start(
                out=enc[k], in_=t["encT"][k * 128 : (k + 1) * 128, :]
            )
        wih_t, whh_t = [], []
        for k in range(2 * NKT):
            wt_ = gws.tile([128, 3 * HL], F32R, tag="wih", name=f"wih{k}")
            nc.scalar.dma_start(out=wt_, in_=t["wihTc"][k * 128 : (k + 1) * 128, :])
            wih_t.append(wt_)
        for k in range(NKT):
            wt_ = gws.tile([128, 3 * HL], F32R, tag="whh", name=f"whh{k}")
            nc.scalar.dma_start(out=wt_, in_=t["whhTc"][k * 128 : (k + 1) * 128, :])
            whh_t.append(wt_)
        # GRU operands that don't depend on attention
        xk = {}
        for k in range(NKT):
            xt_ = pp.tile([128, B], F32R, tag=f"xk{k}", name=f"xk{k}")
            nc.scalar.dma_start(
                out=xt_, in_=t["embTf"][k * 128 : (k + 1) * 128, :]
            )
            xk[k] = xt_
        hfk = []
        for k in range(NKT):
            ht_ = pp.tile([128, B], F32R, tag=f"hfk{k}", name=f"hfk{k}")
            nc.scalar.dma_start(out=ht_, in_=t["hTfull"][k * 128 : (k + 1) * 128, :])
            hfk.append(ht_)
        gb = pp.tile([HL, 4], F32, tag="gb")
        nc.sync.dma_start(out=gb, in_=t["gbias"])
        hc = pp.tile([HL, B], F32, tag="hc")
        nc.sync.dma_start(out=hc, in_=t["hTc"])
        # fc weights: 4 whole-row loads, sliced per chunk at matmul time
        fct = []
        for k in range(NKT):
            ft = fs.tile([128, VS], mybir.dt.bfloat16, tag="fct", name=f"fct{k}")
            nc.scalar.dma_start(out=ft, in_=t["fcT"][k * 128 : (k + 1) * 128, :])
            fct.append(ft)
        fcbt = bs.tile([1, VS], F32R, tag="fcbt")
        nc.scalar.dma_start(out=fcbt, in_=t["fcb"])

        # ---------------- Uh' = U_w @ h.T + (W_b + U_b), transposed --------
        uh = pp.tile([128, NKT, BL], F32, tag="uh")
        uhT = pp.tile([BL, NKT, 128], F32R, tag="uhT")
        with tc.tile_pool(name="psA", bufs=2, space="PSUM") as psA:
            for m in range(NKT):
                pu = psA.tile([128, BL], F32, tag="pu")
                for k in range(NKT):
                    nc.tensor.matmul(
                        pu[:],
                        uw[k][:, m * 128 : (m + 1) * 128],
                        hTt[:, k, :],
                        start=(k == 0),
                        stop=(k == NKT - 1),
                    )
                nc.vector.tensor_scalar_add(uh[:, m, :], pu[:], bwu[:, m : m + 1])
            for m in range(NKT):
                pt_ = psA.tile([BL, 128], F32, tag="ptu")
                nc.tensor.transpose(pt_[:], uh[:, m, :], id128[:])
                nc.vector.tensor_copy(out=uhT[:, m, :], in_=pt_[:])

        # ---------------- energy / scores / softmax / context, per chunk ---
        ctxT = pp.tile([128, NKT, BL], F32R, tag="ctxT")
        with (
            tc.tile_pool(name="psE", bufs=2, space="PSUM") as psE,
            tc.tile_pool(name="psS", bufs=1, space="PSUM") as psS,
            tc.tile_pool(name="psB", bufs=1, space="PSUM") as psB,
            tc.tile_pool(name="attn_sb", bufs=4) as asb,
            tc.tile_pool(name="soft", bufs=2) as sp,
            tc.tile_pool(name="ctx_sb", bufs=2) as csb,
        ):
            for r in range(NRC):
                rsl = slice(r * RC, (r + 1) * RC)
                ps_s = psS.tile([1, RC], F32, tag="ps_s")
                for m in range(NKT):
                    pe = psE.tile([128, RC], F32, tag="pe")
                    for k in range(NKT):
                        nc.tensor.matmul(
                            pe[:],
                            ww[k][:, m * 128 : (m + 1) * 128],
                            enc[k][:, rsl],
                            start=(k == 0),
                            stop=False,
                        )
                    # += Uh'[a, b] broadcast over s via selector matmul
                    nc.tensor.matmul(
                        pe[:], uhT[:, m, :], bsel[:, rsl], start=False, stop=True
                    )
                    tt = asb.tile([128, RC], F32R, tag="tt")
                    nc.scalar.activation(out=tt, in_=pe[:], func=AF.Tanh)
                    nc.tensor.matmul(
                        ps_s[:],
                        vt[:, m : m + 1],
                        tt[:],
                        start=(m == 0),
                        stop=(m == NKT - 1),
                    )
                # segmented softmax on partition 0 (2 batch rows x 256);
                # per-segment max/sum ride ACT's per-partition bias/scale ports
                s3 = ps_s[:].rearrange("p (b s) -> p b s", b=BPC)
                mx = sp.tile([1, BPC, 1], F32, tag="mx")
                nc.vector.tensor_reduce(
                    out=mx, in_=s3, axis=mybir.AxisListType.X, op=OP.max
                )
                mxn = sp.tile([1, BPC, 1], F32, tag="mxn")
                nc.vector.tensor_scalar_mul(mxn[:], mx[:], -1.0)
                ex = sp.tile([1, BPC, S], F32, tag="ex")
                sm = sp.tile([1, BPC, 1], F32, tag="sm")
                rs = sp.tile([1, BPC, 1], F32, tag="rs")
                alpha = sp.tile([1, BPC, S], F32R, tag="alpha")
                for b in range(BPC):
                    nc.scalar.activation(
                        out=ex[:, b, :], in_=s3[:, b, :], func=AF.Exp,
                        bias=mxn[:, b, :], scale=1.0, accum_out=sm[:, b, :],
                    )
                nc.vector.reciprocal(rs[:], sm[:])
                for b in range(BPC):
                    nc.scalar.mul(alpha[:, b, :], ex[:, b, :], rs[:, b, :])
                # broadcast alpha to all 128 partitions via K=1 ones matmul
                ab = psB.tile([128, RC], F32, tag="ab")
                nc.tensor.matmul(
                    ab[:], onest[:], alpha[:].rearrange("p b s -> p (b s)"),
                    start=True, stop=True,
                )
                # context^T chunk: multiply then segmented reduce
                for k in range(NKT):
                    mt = csb.tile([128, BPC, S], F32, tag="mt")
                    nc.vector.tensor_tensor(
                        out=mt,
                        in0=enc[k][:, rsl].rearrange("p (b s) -> p b s", b=BPC),
                        in1=ab[:].rearrange("p (b s) -> p b s", b=BPC),
                        op=OP.mult,
                    )
                    with nc.allow_low_precision(reason="float32r is 32-bit"):
                        nc.vector.tensor_reduce(
                            out=ctxT[:, k, BPC * r : BPC * (r + 1)],
                            in_=mt[:],
                            axis=mybir.AxisListType.X,
                            op=OP.add,
                        )

        # ---------------- AllGather context^T ----------------
        nc.sync.dma_start(
            out=t["cc1_in"].rearrange("(k p) b -> p k b", p=128),
            in_=ctxT[:].bitcast(F32),
        )
        if use_cc:
            _gather(nc, "cc1", t["cc1_in"], t["cc1_out"])
        else:
            for c in range(NCORES):
                nc.sync.dma_start(out=t["cc1_out"][c], in_=t["cc1_in"])
        # ---------------- GRU: full batch, this core's 64 hidden dims ------
        hnc = pp.tile([HL, B], F32, tag="hnc")
        with (
            tc.tile_pool(name="psG", bufs=1, space="PSUM") as psG,
            tc.tile_pool(name="gru_sb", bufs=1) as gsb,
        ):
            def gate_pre(tag, g, with_ih, with_hh, stop_here):
                """Embedded + hidden matmuls: no dependency on the gather."""
                gsl = slice(g * HL, (g + 1) * HL)
                ps_ = psG.tile([HL, B], F32, tag=tag, name=tag)[:]
                ops = []
                if with_ih:
                    ops += [(wih_t[k][:, gsl], xk[k][:]) for k in range(NKT)]
                if with_hh:
                    ops += [(whh_t[k][:, gsl], hfk[k][:]) for k in range(NKT)]
                for i, (lhs, rhs) in enumerate(ops):
                    nc.tensor.matmul(
                        ps_, lhs, rhs,
                        start=(i == 0),
                        stop=(stop_here and i == len(ops) - 1),
                    )
                return ps_

            pr = gate_pre("pr", 0, True, True, False)
            pz = gate_pre("pz", 1, True, True, False)
            pi = gate_pre("pi", 2, True, False, False)
            ph = gate_pre("ph", 2, False, True, True)

            # gathered context -> xcat^T k-tiles 4..7
            xg = pp.tile([128, NKT, NCORES, BL], F32R, tag="xg")
            for k in range(NKT):
                nc.gpsimd.dma_start(
                    out=xg[:, k, :, :],
                    in_=t["cc1_out"].rearrange("c (k p) b -> k p c b", p=128)[k],
                )
            for k in range(NKT, 2 * NKT):
                xk[k] = xg[:, k - NKT, :, :]
            for g, ps_ in ((0, pr), (1, pz), (2, pi)):
                gsl = slice(g * HL, (g + 1) * HL)
                for k in range(NKT, 2 * NKT):
                    nc.tensor.matmul(
                        ps_, wih_t[k][:, gsl], xk[k][:],
                        start=False, stop=(k == 2 * NKT - 1),
                    )

            rg = gsb.tile([HL, B], F32, tag="rg")
            nc.scalar.activation(
                out=rg, in_=pr, func=AF.Sigmoid, bias=gb[:, 0:1], scale=1.0
            )
            zg = gsb.tile([HL, B], F32, tag="zg")
            nc.scalar.activation(
                out=zg, in_=pz, func=AF.Sigmoid, bias=gb[:, 1:2], scale=1.0
            )
            hn_s = gsb.tile([HL, B], F32, tag="hn_s")
            nc.scalar.activation(
                out=hn_s, in_=ph, func=AF.Identity, bias=gb[:, 3:4], scale=1.0
            )
            t1 = gsb.tile([HL, B], F32, tag="t1")
            nc.vector.tensor_tensor(out=t1, in0=rg, in1=hn_s, op=OP.mult)
            t2 = gsb.tile([HL, B], F32, tag="t2")
            nc.vector.tensor_tensor(out=t2, in0=t1, in1=pi, op=OP.add)
            ng = gsb.tile([HL, B], F32, tag="ng")
            nc.scalar.activation(
                out=ng, in_=t2, func=AF.Tanh, bias=gb[:, 2:3], scale=1.0
            )
            # h_new = n + z * (h - n)
            dg = gsb.tile([HL, B], F32, tag="dg")
            nc.vector.tensor_tensor(out=dg, in0=hc, in1=ng, op=OP.subtract)
            zd = gsb.tile([HL, B], F32, tag="zd")
            nc.vector.tensor_tensor(out=zd, in0=zg, in1=dg, op=OP.mult)
            nc.vector.tensor_tensor(out=hnc, in0=ng, in1=zd, op=OP.add)

        nc.sync.dma_start(out=t["cc2_in"], in_=hnc[:])
        if use_cc:
            _gather(nc, "cc2", t["cc2_in"], t["cc2_out"])
        else:
            for c in range(NCORES):
                nc.sync.dma_start(
                    out=t["cc2_out"][c * HL : (c + 1) * HL, :], in_=t["cc2_in"]
                )
        nc.sync.dma_start(out=t["hnewT"], in_=t["cc2_out"])

        # ---------------- fc (vocab shard, full batch) ----------------
        with (
            tc.tile_pool(name="psF", bufs=3, space="PSUM") as psF,
            tc.tile_pool(name="fc_sb", bufs=1) as fsb,
        ):
            hg = fsb.tile([128, NKT, B], mybir.dt.bfloat16, tag="hg")
            nc.gpsimd.dma_start(
                out=hg, in_=t["cc2_out"].rearrange("(k p) b -> p k b", p=128)
            )
            hTf = [hg[:, k, :] for k in range(NKT)]
            lgall = fsb.tile([B, VS], F32, tag="lgall")
            for j in range(NFC):
                jsl = slice(j * FCN, (j + 1) * FCN)
                pf = psF.tile([B, FCN], F32, tag="pf")
                nc.tensor.matmul(
                    pf[:], onest[:, :B], fcbt[:, jsl], start=True, stop=False,
                )
                for k in range(NKT):
                    nc.tensor.matmul(
                        pf[:], hTf[k][:], fct[k][:, jsl],
                        start=False, stop=(k == NKT - 1),
                    )
                nc.vector.tensor_copy(out=lgall[:, jsl], in_=pf[:])
                if j == NFC // 2 - 1:  # first half streams out during 2nd half
                    nc.sync.dma_start(
                        out=t["logits"][:, : VS // 2], in_=lgall[:, : VS // 2]
                    )
            nc.sync.dma_start(
                out=t["logits"][:, VS // 2 :], in_=lgall[:, VS // 2 :]
            )


def build_program(unroll=1, use_cc=True, num_devices=NCORES):
    nc = bacc.Bacc(
        "TRN2", target_bir_lowering=False, debug=False, num_devices=num_devices
    )
    t = _declare_io(nc)
    if unroll != 1 or not use_cc:
        # Shape-varying dummy input: makes the HLO module signature unique per
        # variant so the NEFF compile cache cannot alias them.
        nc.dram_tensor(
            "utick", [1, unroll + (0 if use_cc else 100)], F32, kind="ExternalInput"
        )
    with tile.TileContext(nc) as tc:
        for _ in range(unroll):
            _emit(nc, tc, t, use_cc=use_cc)
    nc.compile()
    return nc


def prepare_in_maps(inputs):
    """Shard + lay out the full inputs into 8 per-core input dicts."""
    f = np.float32
    x = np.asarray(inputs["x"])
    h = np.asarray(inputs["hidden"], dtype=f)[0]              # [B, H]
    enc = np.asarray(inputs["encoder_outputs"], dtype=f)      # [B, S, E]
    embedded = np.asarray(inputs["emb_table"], dtype=f)[x]    # [B, E]

    def C(a):
        return np.ascontiguousarray(a, dtype=f)

    wihT = np.asarray(inputs["W_ih"], dtype=f).T              # [1024, 1536]
    whhT = np.asarray(inputs["W_hh"], dtype=f).T              # [512, 1536]
    bih = np.asarray(inputs["b_ih"], dtype=f)
    bhh = np.asarray(inputs["b_hh"], dtype=f)
    hT = C(h.T)

    shared = {
        "wuT": np.ascontiguousarray(
            np.concatenate(
                [np.asarray(inputs["W_w"]).T, np.asarray(inputs["U_w"]).T], axis=1
            ).astype(ml_dtypes.bfloat16)
        ),
        "vw": C(np.asarray(inputs["V_w"]).reshape(1, A)),
        "wub": C(
            (np.asarray(inputs["W_b"]) + np.asarray(inputs["U_b"])).reshape(1, A)
        ),
        "hTfull": hT,
        "embTf": C(embedded.T),
        "bsel": C(np.repeat(np.eye(BL, dtype=f), S, axis=1)),
        "id128": np.eye(128, dtype=f),
        "ones": np.ones((1, 128), dtype=f),
    }
    fc_w = np.asarray(inputs["fc_w"], dtype=f)
    fc_b = np.asarray(inputs["fc_b"], dtype=f)

    in_maps = []
    for c in range(NCORES):
        bs_ = slice(c * BL, (c + 1) * BL)
        vs_ = slice(c * VS, (c + 1) * VS)
        hs = [slice(g * H + c * HL, g * H + (c + 1) * HL) for g in range(3)]
        m = dict(shared)
        m["encT"] = np.ascontiguousarray(enc[bs_].reshape(R, E).T.astype(ml_dtypes.bfloat16))
        m["hT"] = np.ascontiguousarray(h[bs_].T.astype(ml_dtypes.bfloat16))
        m["hTc"] = C(hT[c * HL : (c + 1) * HL, :])
        m["wihTc"] = C(np.concatenate([wihT[:, s] for s in hs], axis=1))
        m["whhTc"] = C(np.concatenate([whhT[:, s] for s in hs], axis=1))
        m["gbias"] = C(
            np.stack(
                [
                    bih[hs[0]] + bhh[hs[0]],
                    bih[hs[1]] + bhh[hs[1]],
                    bih[hs[2]],
                    bhh[hs[2]],
                ],
                axis=1,
            )
        )
        m["fcT"] = np.ascontiguousarray(fc_w[vs_].T.astype(ml_dtypes.bfloat16))
        m["fcb"] = C(fc_b[vs_].reshape(1, VS))
        in_maps.append(m)
    return in_maps


def assemble(results):
    logits = np.concatenate([results[c]["logits"] for c in range(NCORES)], axis=1)
    hnew = results[0]["hnewT"].T
    return logits, hnew[None]


_CACHED_NC = None


def kernel(**inputs):
    global _CACHED_NC
    from concourse.bass_utils import run_bass_kernel_spmd

    if _CACHED_NC is None:
        _CACHED_NC = build_program()
    in_maps = prepare_in_maps(inputs)
    res = run_bass_kernel_spmd(_CACHED_NC, in_maps, list(range(NCORES)))
    return assemble(res.results)


# revision 53
# speedup vs baseline: 1.2618x; 1.0901x over previous
"""Trainium2 Bass kernel for a single-step attention GRU decoder.

Model (per reference):
    embedded = emb_table[x]                               # [B, E]
    energy   = tanh(enc @ W_w.T + W_b + (h @ U_w.T + U_b)[:, None, :])
    scores   = energy @ V_w[0] + V_b
    alpha    = softmax(scores, axis=S)
    context  = alpha @ enc                                # [B, E]
    GRU single step on [embedded, context] -> h_new       # [B, H]
    prediction = h_new @ fc_w.T + fc_b                    # [B, V]

Sharding (8 NeuronCores):
  - Attention is data-parallel over batch (8 rows/core); the encoder slice
    ships pre-transposed ([E, B_loc*S]) so contractions sit on partitions.
    The U_w@h term (plus biases) is folded into the energy PSUM via a
    selector matmul; softmax+context are pipelined per 512-row chunk.
  - context^T shards are AllGathered (16 KB); the GRU is tensor-parallel
    over hidden dims (full batch, 64 of 512 dims per core), so each core
    loads only 1/8 of W_ih/W_hh.  Gate biases ride the activations'
    per-partition bias port.
  - h_new^T shards are AllGathered; the fc layer is tensor-parallel over
    vocab (all 64 batch rows x 4000 vocab rows per core); the host
    concatenates logit shards.
  - Host-side prep: embedding gather (64 rows), weight transposes,
    sharding.  All FLOPs run on-device; matmuls use fp32r.
"""

import os
import sys

import numpy as np

try:
    import ml_dtypes
except ImportError:  # bf16 numpy dtype
    ml_dtypes = None

if "/opt/trn_rl_repo" not in sys.path:
    sys.path.insert(0, "/opt/trn_rl_repo")

import concourse.bass as bass  # noqa: E402
import concourse.tile as tile  # noqa: E402
from concourse import bacc, mybir  # noqa: E402

F32 = mybir.dt.float32
F32R = mybir.dt.float32r
AF = mybir.ActivationFunctionType
OP = mybir.AluOpType

NCORES = 8
B, S, E, H, A, V = 64, 256, 512, 512, 512, 32000
BL = B // NCORES          # 8 batch rows per core
HL = H // NCORES          # 64 hidden dims per core (GRU shard)
VS = V // NCORES          # 4000 vocab rows per core
R = BL * S                # 2048 attention rows per core
G3 = 3 * H                # 1536
NKT = E // 128            # 4 k-tiles per 512-dim contraction
RC = 512                  # row-chunk (free dim) for the energy matmul
NRC = R // RC             # 4 row chunks
BPC = RC // S             # 2 batch rows per row chunk
FCN = 500                 # fc free-dim chunk
NFC = VS // FCN           # 8 fc chunks


def _declare_io(nc):
    t = {}

    def inp(name, shape, dt=F32R):
        t[name] = nc.dram_tensor(name, list(shape), dt, kind="ExternalInput").ap()

    def outp(name, shape, dt=F32):
        t[name] = nc.dram_tensor(name, list(shape), dt, kind="ExternalOutput").ap()

    inp("encT", (E, R), mybir.dt.bfloat16)      # encoder slice, transposed (bf16)
    inp("wuT", (E, 2 * A), mybir.dt.bfloat16)   # [W_w.T | U_w.T] (bf16)
    inp("vw", (1, A))              # V_w
    inp("wub", (1, A), F32)        # W_b + U_b
    inp("hT", (H, BL), mybir.dt.bfloat16)       # local batch hidden, transposed (bf16)
    inp("hTfull", (H, B))          # full-batch hidden, transposed (GRU gh)
    inp("hTc", (HL, B), F32)       # hidden slice for this core's GRU dims
    inp("embTf", (E, B))           # full embedded rows, transposed
    inp("wihTc", (E + E, 3 * HL))  # W_ih.T cols for this core's dims [1024,192]
    inp("whhTc", (H, 3 * HL))      # W_hh.T cols likewise [512, 192]
    inp("gbias", (HL, 4), F32)     # [r:bih+bhh, z:bih+bhh, n:bih, n:bhh]
    inp("fcT", (H, VS), mybir.dt.bfloat16)  # local fc_w slice, transposed (bf16)
    inp("fcb", (1, VS))            # local fc_b slice
    inp("ones", (1, 128))          # ones for K=1 broadcast matmuls

    outp("logits", (B, VS))
    outp("hnewT", (H, B))

    # collective buffers
    t["cc1_in"] = nc.dram_tensor("cc1_in", [E, BL], F32).ap()
    t["cc1_out"] = nc.dram_tensor(
        "cc1_out", [NCORES, E, BL], F32, addr_space="Shared"
    ).ap()
    t["cc2_in"] = nc.dram_tensor("cc2_in", [HL, B], F32).ap()
    t["cc2_out"] = nc.dram_tensor(
        "cc2_out", [H, B], F32, addr_space="Shared"
    ).ap()
    return t


def _gather(nc, kind, t_in, t_out):
    nc.gpsimd.collective_compute(
        "AllGather",
        OP.bypass,
        replica_groups=[list(range(NCORES))],
        ins=[t_in],
        outs=[t_out],
    )


def _emit(nc, tc, t, use_cc=True):
    with (
        tc.tile_pool(name="persist", bufs=1) as pp,
        tc.tile_pool(name="fcstream", bufs=4) as fs,
        tc.tile_pool(name="bias1", bufs=1) as bs,
        tc.tile_pool(name="gru_w", bufs=12) as gws,
    ):
        # ------- small persistent loads first (unblock PE quickly) -------
        wu = [
            pp.tile([128, 2 * A], mybir.dt.bfloat16, tag=f"wu{k}", name=f"wu{k}")
            for k in range(NKT)
        ]
        for k in range(NKT):
            nc.sync.dma_start(out=wu[k], in_=t["wuT"][k * 128 : (k + 1) * 128, :])
        ww = [w[:, :A] for w in wu]
        uw = [w[:, A:] for w in wu]
        vt = pp.tile([128, NKT], F32R, tag="vt")
        nc.sync.dma_start(out=vt, in_=t["vw"].rearrange("o (m p) -> (o p) m", p=128))
        hTt = pp.tile([128, NKT, BL], mybir.dt.bfloat16, tag="hTt")
        nc.sync.dma_start(out=hTt, in_=t["hT"].rearrange("(k p) b -> p k b", p=128))
        onest = pp.tile([1, 128], F32R, tag="ones")
        nc.sync.dma_start(out=onest, in_=t["ones"])
        bwu = pp.tile([128, NKT], F32, tag="bwu")
        nc.sync.dma_start(out=bwu, in_=t["wub"].rearrange("o (m p) -> (o p) m", p=128))
        # V_b shifts every score equally -> softmax-invariant; not loaded.

        # ------- bulk loads on the ACT HWDGE queue (few, large) -----------
        enc = [
            pp.tile([128, R], mybir.dt.bfloat16, tag=f"enc{k}", name=f"enc{k}")
            for k in range(NKT)
        ]
        for k in range(NKT):
            nc.scalar.dma_start(
                out=enc[k], in_=t["encT"][k * 128 : (k + 1) * 128, :]
            )
        wih_t, whh_t = [], []
        for k in range(2 * NKT):
            wt_ = gws.tile([128, 3 * HL], F32R, tag="wih", name=f"wih{k}")
            nc.scalar.dma_start(out=wt_, in_=t["wihTc"][k * 128 : (k + 1) * 128, :])
            wih_t.append(wt_)
        for k in range(NKT):
            wt_ = gws.tile([128, 3 * HL], F32R, tag="whh", name=f"whh{k}")
            nc.scalar.dma_start(out=wt_, in_=t["whhTc"][k * 128 : (k + 1) * 128, :])
            whh_t.append(wt_)
        # GRU operands that don't depend on attention
        xk = {}
        for k in range(NKT):
            xt_ = pp.tile([128, B], F32R, tag=f"xk{k}", name=f"xk{k}")
            nc.scalar.dma_start(
                out=xt_, in_=t["embTf"][k * 128 : (k + 1) * 128, :]
            )
            xk[k] = xt_
        hfk = []
        for k in range(NKT):
            ht_ = pp.tile([128, B], F32R, tag=f"hfk{k}", name=f"hfk{k}")
            nc.scalar.dma_start(out=ht_, in_=t["hTfull"][k * 128 : (k + 1) * 128, :])
            hfk.append(ht_)
        gb = pp.tile([HL, 4], F32, tag="gb")
        nc.sync.dma_start(out=gb, in_=t["gbias"])
        hc = pp.tile([HL, B], F32, tag="hc")
        nc.sync.dma_start(out=hc, in_=t["hTc"])
        # fc weights: 4 whole-row loads, sliced per chunk at matmul time
        fct = []
        for k in range(NKT):
            ft = fs.tile([128, VS], mybir.dt.bfloat16, tag="fct", name=f"fct{k}")
            nc.scalar.dma_start(out=ft, in_=t["fcT"][k * 128 : (k + 1) * 128, :])
            fct.append(ft)
        fcbt = bs.tile([1, VS], F32R, tag="fcbt")
        nc.scalar.dma_start(out=fcbt, in_=t["fcb"])

        # ------- Uh' = U_w @ h.T + (W_b + U_b): rides tanh's bias port -----
        uh = pp.tile([128, NKT, BL], F32, tag="uh")
        with tc.tile_pool(name="psA", bufs=2, space="PSUM") as psA:
            for m in range(NKT):
                pu = psA.tile([128, BL], F32, tag="pu")
                for k in range(NKT):
                    nc.tensor.matmul(
                        pu[:],
                        uw[k][:, m * 128 : (m + 1) * 128],
                        hTt[:, k, :],
                        start=(k == 0),
                        stop=(k == NKT - 1),
                    )
                nc.vector.tensor_scalar_add(uh[:, m, :], pu[:], bwu[:, m : m + 1])

        # ---------------- energy / scores / softmax / context, per chunk ---
        ctxT = pp.tile([128, NKT, BL], F32R, tag="ctxT")
        with (
            tc.tile_pool(name="psE", bufs=2, space="PSUM") as psE,
            tc.tile_pool(name="psS", bufs=1, space="PSUM") as psS,
            tc.tile_pool(name="psB", bufs=1, space="PSUM") as psB,
            tc.tile_pool(name="attn_sb", bufs=4) as asb,
            tc.tile_pool(name="soft", bufs=2) as sp,
            tc.tile_pool(name="ctx_sb", bufs=2) as csb,
        ):
            for r in range(NRC):
                rsl = slice(r * RC, (r + 1) * RC)
                ps_s = psS.tile([1, RC], F32, tag="ps_s")
                for m in range(NKT):
                    pe = psE.tile([128, RC], F32, tag="pe")
                    for k in range(NKT):
                        nc.tensor.matmul(
                            pe[:],
                            ww[k][:, m * 128 : (m + 1) * 128],
                            enc[k][:, rsl],
                            start=(k == 0),
                            stop=(k == NKT - 1),
                        )
                    # tanh(W@enc + Uh'[a, b]): Uh rides the per-partition
                    # bias port, one ACT call per batch row in the chunk
                    tt = asb.tile([128, RC], F32R, tag="tt")
                    for b in range(BPC):
                        nc.scalar.activation(
                            out=tt[:, b * S : (b + 1) * S],
                            in_=pe[:, b * S : (b + 1) * S],
                            func=AF.Tanh,
                            bias=uh[:, m, BPC * r + b : BPC * r + b + 1],
                            scale=1.0,
                        )
                    nc.tensor.matmul(
                        ps_s[:],
                        vt[:, m : m + 1],
                        tt[:],
                        start=(m == 0),
                        stop=(m == NKT - 1),
                    )
                # segmented softmax on partition 0 (2 batch rows x 256);
                # per-segment max/sum ride ACT's per-partition bias/scale ports
                s3 = ps_s[:].rearrange("p (b s) -> p b s", b=BPC)
                mx = sp.tile([1, BPC, 1], F32, tag="mx")
                nc.vector.tensor_reduce(
                    out=mx, in_=s3, axis=mybir.AxisListType.X, op=OP.max
                )
                mxn = sp.tile([1, BPC, 1], F32, tag="mxn")
                nc.vector.tensor_scalar_mul(mxn[:], mx[:], -1.0)
                ex = sp.tile([1, BPC, S], F32, tag="ex")
                sm = sp.tile([1, BPC, 1], F32, tag="sm")
                rs = sp.tile([1, BPC, 1], F32, tag="rs")
                alpha = sp.tile([1, BPC, S], F32R, tag="alpha")
                for b in range(BPC):
                    nc.scalar.activation(
                        out=ex[:, b, :], in_=s3[:, b, :], func=AF.Exp,
                        bias=mxn[:, b, :], scale=1.0, accum_out=sm[:, b, :],
                    )
                nc.vector.reciprocal(rs[:], sm[:])
                for b in range(BPC):
                    nc.scalar.mul(alpha[:, b, :], ex[:, b, :], rs[:, b, :])
                # broadcast alpha to all 128 partitions via K=1 ones matmul
                ab = psB.tile([128, RC], F32, tag="ab")
                nc.tensor.matmul(
                    ab[:], onest[:], alpha[:].rearrange("p b s -> p (b s)"),
                    start=True, stop=True,
                )
                # context^T chunk: multiply then segmented reduce
                for k in range(NKT):
                    mt = csb.tile([128, BPC, S], F32, tag="mt")
                    nc.vector.tensor_tensor(
                        out=mt,
                        in0=enc[k][:, rsl].rearrange("p (b s) -> p b s", b=BPC),
                        in1=ab[:].rearrange("p (b s) -> p b s", b=BPC),
                        op=OP.mult,
                    )
                    with nc.allow_low_precision(reason="float32r is 32-bit"):
                        nc.vector.tensor_reduce(
                            out=ctxT[:, k, BPC * r : BPC * (r + 1)],
                            in_=mt[:],
                            axis=mybir.AxisListType.X,
                            op=OP.add,
                        )

        # ---------------- AllGather context^T ----------------
        nc.sync.dma_start(
            out=t["cc1_in"].rearrange("(k p) b -> p k b", p=128),
            in_=ctxT[:].bitcast(F32),
        )
        if use_cc:
            _gather(nc, "cc1", t["cc1_in"], t["cc1_out"])
        else:
            for c in range(NCORES):
                nc.sync.dma_start(out=t["cc1_out"][c], in_=t["cc1_in"])
        # ---------------- GRU: full batch, this core's 64 hidden dims ------
        hnc = pp.tile([HL, B], F32, tag="hnc")
        with (
            tc.tile_pool(name="psG", bufs=1, space="PSUM") as psG,
            tc.tile_pool(name="gru_sb", bufs=1) as gsb,
        ):
            def gate_pre(tag, g, with_ih, with_hh, stop_here):
                """Embedded + hidden matmuls: no dependency on the gather."""
                gsl = slice(g * HL, (g + 1) * HL)
                ps_ = psG.tile([HL, B], F32, tag=tag, name=tag)[:]
                ops = []
                if with_ih:
                    ops += [(wih_t[k][:, gsl], xk[k][:]) for k in range(NKT)]
                if with_hh:
                    ops += [(whh_t[k][:, gsl], hfk[k][:]) for k in range(NKT)]
                for i, (lhs, rhs) in enumerate(ops):
                    nc.tensor.matmul(
                        ps_, lhs, rhs,
                        start=(i == 0),
                        stop=(stop_here and i == len(ops) - 1),
                    )
                return ps_

            pr = gate_pre("pr", 0, True, True, False)
            pz = gate_pre("pz", 1, True, True, False)
            pi = gate_pre("pi", 2, True, False, False)
            ph = gate_pre("ph", 2, False, True, True)

            # gathered context -> xcat^T k-tiles 4..7
            xg = pp.tile([128, NKT, NCORES, BL], F32R, tag="xg")
            for k in range(NKT):
                nc.gpsimd.dma_start(
                    out=xg[:, k, :, :],
                    in_=t["cc1_out"].rearrange("c (k p) b -> k p c b", p=128)[k],
                )
            for k in range(NKT, 2 * NKT):
                xk[k] = xg[:, k - NKT, :, :]
            for g, ps_ in ((0, pr), (1, pz), (2, pi)):
                gsl = slice(g * HL, (g + 1) * HL)
                for k in range(NKT, 2 * NKT):
                    nc.tensor.matmul(
                        ps_, wih_t[k][:, gsl], xk[k][:],
                        start=False, stop=(k == 2 * NKT - 1),
                    )

            rg = gsb.tile([HL, B], F32, tag="rg")
            nc.scalar.activation(
                out=rg, in_=pr, func=AF.Sigmoid, bias=gb[:, 0:1], scale=1.0
            )
            zg = gsb.tile([HL, B], F32, tag="zg")
            nc.scalar.activation(
                out=zg, in_=pz, func=AF.Sigmoid, bias=gb[:, 1:2], scale=1.0
            )
            hn_s = gsb.tile([HL, B], F32, tag="hn_s")
            nc.scalar.activation(
                out=hn_s, in_=ph, func=AF.Identity, bias=gb[:, 3:4], scale=1.0
            )
            t1 = gsb.tile([HL, B], F32, tag="t1")
            nc.vector.tensor_tensor(out=t1, in0=rg, in1=hn_s, op=OP.mult)
            t2 = gsb.tile([HL, B], F32, tag="t2")
            nc.vector.tensor_tensor(out=t2, in0=t1, in1=pi, op=OP.add)
            ng = gsb.tile([HL, B], F32, tag="ng")
            nc.scalar.activation(
                out=ng, in_=t2, func=AF.Tanh, bias=gb[:, 2:3], scale=1.0
            )
            # h_new = n + z * (h - n)
            dg = gsb.tile([HL, B], F32, tag="dg")
            nc.vector.tensor_tensor(out=dg, in0=hc, in1=ng, op=OP.subtract)
            zd = gsb.tile([HL, B], F32, tag="zd")
            nc.vector.tensor_tensor(out=zd, in0=zg, in1=dg, op=OP.mult)
            nc.vector.tensor_tensor(out=hnc, in0=ng, in1=zd, op=OP.add)

        nc.sync.dma_start(out=t["cc2_in"], in_=hnc[:])
        if use_cc:
            _gather(nc, "cc2", t["cc2_in"], t["cc2_out"])
        else:
            for c in range(NCORES):
                nc.sync.dma_start(
                    out=t["cc2_out"][c * HL : (c + 1) * HL, :], in_=t["cc2_in"]
                )
        nc.sync.dma_start(out=t["hnewT"], in_=t["cc2_out"])

        # ---------------- fc (vocab shard, full batch) ----------------
        with (
            tc.tile_pool(name="psF", bufs=3, space="PSUM") as psF,
            tc.tile_pool(name="fc_sb", bufs=1) as fsb,
        ):
            hg = fsb.tile([128, NKT, B], mybir.dt.bfloat16, tag="hg")
            nc.gpsimd.dma_start(
                out=hg, in_=t["cc2_out"].rearrange("(k p) b -> p k b", p=128)
            )
            hTf = [hg[:, k, :] for k in range(NKT)]
            lgall = fsb.tile([B, VS], F32, tag="lgall")
            for j in range(NFC):
                jsl = slice(j * FCN, (j + 1) * FCN)
                pf = psF.tile([B, FCN], F32, tag="pf")
                nc.tensor.matmul(
                    pf[:], onest[:, :B], fcbt[:, jsl], start=True, stop=False,
                )
                for k in range(NKT):
                    nc.tensor.matmul(
                        pf[:], hTf[k][:], fct[k][:, jsl],
                        start=False, stop=(k == NKT - 1),
                    )
                nc.vector.tensor_copy(out=lgall[:, jsl], in_=pf[:])
                if j == NFC // 2 - 1:  # first half streams out during 2nd half
                    nc.sync.dma_start(
                        out=t["logits"][:, : VS // 2], in_=lgall[:, : VS // 2]
                    )
            nc.sync.dma_start(
                out=t["logits"][:, VS // 2 :], in_=lgall[:, VS // 2 :]
            )


def build_program(unroll=1, use_cc=True, num_devices=NCORES):
    nc = bacc.Bacc(
        "TRN2", target_bir_lowering=False, debug=False, num_devices=num_devices
    )
    t = _declare_io(nc)
    if unroll != 1 or not use_cc:
        # Shape-varying dummy input: makes the HLO module signature unique per
        # variant so the NEFF compile cache cannot alias them.
        nc.dram_tensor(
            "utick", [1, unroll + (0 if use_cc else 100)], F32, kind="ExternalInput"
        )
    with tile.TileContext(nc) as tc:
        for _ in range(unroll):
            _emit(nc, tc, t, use_cc=use_cc)
    nc.compile()
    return nc


def prepare_in_maps(inputs):
    """Shard + lay out the full inputs into 8 per-core input dicts."""
    f = np.float32
    x = np.asarray(inputs["x"])
    h = np.asarray(inputs["hidden"], dtype=f)[0]              # [B, H]
    enc = np.asarray(inputs["encoder_outputs"], dtype=f)      # [B, S, E]
    embedded = np.asarray(inputs["emb_table"], dtype=f)[x]    # [B, E]

    def C(a):
        return np.ascontiguousarray(a, dtype=f)

    wihT = np.asarray(inputs["W_ih"], dtype=f).T              # [1024, 1536]
    whhT = np.asarray(inputs["W_hh"], dtype=f).T              # [512, 1536]
    bih = np.asarray(inputs["b_ih"], dtype=f)
    bhh = np.asarray(inputs["b_hh"], dtype=f)
    hT = C(h.T)

    shared = {
        "wuT": np.ascontiguousarray(
            np.concatenate(
                [np.asarray(inputs["W_w"]).T, np.asarray(inputs["U_w"]).T], axis=1
            ).astype(ml_dtypes.bfloat16)
        ),
        "vw": C(np.asarray(inputs["V_w"]).reshape(1, A)),
        "wub": C(
            (np.asarray(inputs["W_b"]) + np.asarray(inputs["U_b"])).reshape(1, A)
        ),
        "hTfull": hT,
        "embTf": C(embedded.T),
        "ones": np.ones((1, 128), dtype=f),
    }
    fc_w = np.asarray(inputs["fc_w"], dtype=f)
    fc_b = np.asarray(inputs["fc_b"], dtype=f)

    in_maps = []
    for c in range(NCORES):
        bs_ = slice(c * BL, (c + 1) * BL)
        vs_ = slice(c * VS, (c + 1) * VS)
        hs = [slice(g * H + c * HL, g * H + (c + 1) * HL) for g in range(3)]
        m = dict(shared)
        m["encT"] = np.ascontiguousarray(enc[bs_].reshape(R, E).T.astype(ml_dtypes.bfloat16))
        m["hT"] = np.ascontiguousarray(h[bs_].T.astype(ml_dtypes.bfloat16))
        m["hTc"] = C(hT[c * HL : (c + 1) * HL, :])
        m["wihTc"] = C(np.concatenate([wihT[:, s] for s in hs], axis=1))
        m["whhTc"] = C(np.concatenate([whhT[:, s] for s in hs], axis=1))
        m["gbias"] = C(
            np.stack(
                [
                    bih[hs[0]] + bhh[hs[0]],
                    bih[hs[1]] + bhh[hs[1]],
                    bih[hs[2]],
                    bhh[hs[2]],
                ],
                axis=1,
            )
        )
        m["fcT"] = np.ascontiguousarray(fc_w[vs_].T.astype(ml_dtypes.bfloat16))
        m["fcb"] = C(fc_b[vs_].reshape(1, VS))
        in_maps.append(m)
    return in_maps


def assemble(results):
    logits = np.concatenate([results[c]["logits"] for c in range(NCORES)], axis=1)
    hnew = results[0]["hnewT"].T
    return logits, hnew[None]


_CACHED_NC = None


def kernel(**inputs):
    global _CACHED_NC
    from concourse.bass_utils import run_bass_kernel_spmd

    if _CACHED_NC is None:
        _CACHED_NC = build_program()
    in_maps = prepare_in_maps(inputs)
    res = run_bass_kernel_spmd(_CACHED_NC, in_maps, list(range(NCORES)))
    return assemble(res.results)
